# revision 3
# baseline (speedup 1.0000x reference)
"""Multi-head self-attention (B=4, T=2048, C=768, H=12) on 8 trn2 NeuronCores. v10.

Sharding: core c -> batch b=c//2, head-group g=c%2 (6 heads each).
Host sums the 2 partials per batch and adds the bias.

v10 (from v9): the attention steady state was gated by exp latency (ctx_j
waited ~1us on the ACT exp of chunk j; PE in-order queue head-of-line
blocked, DVFS dropped to mid p-state).
  - skew-2 emission: scores_{j+2} is emitted before ctx_j, so by the time
    the PE reaches ctx_j its exp has had ~2 iterations to finish.
  - one shared PSUM pool `ps` (3 bufs x [128,1024] f32 = 6 banks) serves
    scores chunks, projections, v chunks, transposes and outproj; cps
    keeps the last 2 banks. 3 score chunks in flight make the skew real.
  - exp alternates ACT/DVE (~60/40) via a fractional accumulator instead
    of the fixed 17% Schraudolph offload.
  - gpsimd (Pool engine, SBUF-only) absorbs x/weight casts and the
    normalize multiplies; DVE keeps PSUM-touching copies + its exp share.
  - cps released fast at block ends: cu copy halves go to DVE + ACT
    concurrently; proj/outproj bursts are emitted right after s0/s1 of
    the next block so the exp engines drain the backlog meanwhile.
"""
import sys
import os

sys.path.insert(0, "/opt/trn_rl_repo")

import numpy as np

P = 128
T = 2048
C = 768
HD = 384          # per-core head columns (6 heads x 64)
D = 64
NT = T // P       # 16 key chunks of 128
KC = C // P       # 6 contraction chunks for C
MC = HD // P      # 3 head pairs
QB = 512          # query block
NQ = T // QB      # 4 query blocks

EXP_A = 128 * 0.125 * float(np.log2(np.e))   # 23.083120654223414
EXP_B = 128 * 127 - 5.5                      # 16250.5 (Schraudolph bias, bf16)

# DVE share of exp chunks per phase (fractional accumulator)
F_DVE_LEAD = 0.50    # block (0,0): streams during the PE-heavy lead-in
F_DVE_MAIN = 0.42    # blocks m=0 u>=1 .. m=1
F_DVE_OUT = 0.34     # m=2 phase: DVE also does outproj copies

_cache = {}


def _build(repeat=1):
    import concourse.bacc as bacc
    import concourse.mybir as mybir
    import concourse.tile as tile
    from concourse.masks import make_identity
    from contextlib import ExitStack

    F32 = mybir.dt.float32
    BF16 = mybir.dt.bfloat16
    I16 = mybir.dt.int16
    AF = mybir.ActivationFunctionType
    ALU = mybir.AluOpType

    nc = bacc.Bacc("TRN2", target_bir_lowering=False, debug=False)
    x = nc.dram_tensor("x", [T, C], F32, kind="ExternalInput").ap()
    wq = nc.dram_tensor("wq", [C, HD], F32, kind="ExternalInput").ap()
    wk = nc.dram_tensor("wk", [C, HD], F32, kind="ExternalInput").ap()
    wv = nc.dram_tensor("wv", [C, HD], F32, kind="ExternalInput").ap()
    wo = nc.dram_tensor("wo", [HD, C], F32, kind="ExternalInput").ap()
    out = nc.dram_tensor("out", [T, C], F32, kind="ExternalOutput").ap()

    def emit(pfx, tc, pools):
        ident_bf, big, wrp, vap, work, outp, norm = pools

        xt = [big.tile([P, T], BF16, name=f"{pfx}xt{kc}", tag="big2048") for kc in range(KC)]
        qT = [big.tile([P, T], BF16, name=f"{pfx}qT{m}", tag="big2048") for m in range(MC)]
        kT = [big.tile([P, T], BF16, name=f"{pfx}kT{m}", tag="big2048") for m in range(MC)]
        ctxT = [big.tile([P, T], BF16, name=f"{pfx}ctxT{m}", tag="big2048") for m in range(MC)]
        va = [[[vap.tile([P, D + 1], BF16, name=f"{pfx}va{m}_{t}_{g}", tag=f"va{m}_{t}_{g}")
                for g in range(2)] for t in range(NT)] for m in range(MC)]

        w_b = {}
        wo_b = []
        exp_acc = [0.0]

        def emit_weight_loads(wstage, names):
            for nm, src in names:
                if nm == "o":
                    for m in range(MC):
                        st = wstage.tile([P, C], F32, name=f"{pfx}wst_o{m}", tag="wsto")
                        nc.sync.dma_start(st[:], wo[P * m:P * (m + 1), :])
                        t_b = wrp.tile([P, C], BF16, name=f"{pfx}wo_{m}", tag=f"wo_{m}")
                        nc.gpsimd.tensor_copy(t_b[:], st[:])
                        wo_b.append(t_b)
                    continue
                for kc in range(KC):
                    st = wstage.tile([P, HD], F32, name=f"{pfx}wst_{nm}{kc}", tag="wst")
                    nc.sync.dma_start(st[:], src[P * kc:P * (kc + 1), :])
                    t_b = wrp.tile([P, HD], BF16, name=f"{pfx}w_{nm}{kc}", tag=f"w_{nm}{kc}")
                    nc.gpsimd.tensor_copy(t_b[:], st[:])
                    w_b[nm, kc] = t_b

        def emit_xt_group(tq, xrp, xbp, ps):
            xbs = []
            for i in range(4):
                t_i = 4 * tq + i
                xr = xrp.tile([P, C], F32, name=f"{pfx}xr{t_i}", tag="xr")
                nc.sync.dma_start(xr[:], x[P * t_i:P * (t_i + 1), :])
                xb = xbp.tile([P, C], BF16, name=f"{pfx}xb{t_i}", tag="xb")
                nc.gpsimd.tensor_copy(xb[:], xr[:])
                xbs.append(xb)
            for kc in range(KC):
                tp = ps.tile([P, 512], BF16, name=f"{pfx}tp_{tq}_{kc}", tag="ps")
                for i in range(4):
                    nc.tensor.transpose(tp[:, P * i:P * (i + 1)],
                                        xbs[i][:, P * kc:P * (kc + 1)], ident_bf[:])
                nc.vector.tensor_copy(xt[kc][:, 512 * tq:512 * (tq + 1)], tp[:])

        def emit_proj_block(nm, m, n, ps):
            dest = qT if nm == "q" else kT
            pst = ps.tile([P, 512], F32, name=f"{pfx}ps_{nm}{m}{n}", tag="ps")
            for kc in range(KC):
                nc.tensor.matmul(
                    pst[:],
                    w_b[nm, kc][:, P * m:P * (m + 1)],
                    xt[kc][:, 512 * n:512 * (n + 1)],
                    start=(kc == 0), stop=(kc == KC - 1),
                )
            if m == 0:
                nc.scalar.copy(dest[m][:, 512 * n:512 * (n + 1)], pst[:])
            else:
                nc.vector.tensor_copy(dest[m][:, 512 * n:512 * (n + 1)], pst[:])

        def emit_v_chunk(t_i, ps):
            pv = ps.tile([P, HD], F32, name=f"{pfx}pv{t_i}", tag="ps")
            for kc in range(KC):
                nc.tensor.matmul(
                    pv[:],
                    xt[kc][:, P * t_i:P * (t_i + 1)],
                    w_b["v", kc][:],
                    start=(kc == 0), stop=(kc == KC - 1),
                )
            for m in range(MC):
                for g in range(2):
                    vt = va[m][t_i][g]
                    nc.vector.tensor_copy(vt[:, 0:D], pv[:, P * m + D * g:P * m + D * (g + 1)])
                    nc.gpsimd.memset(vt[:, D:D + 1], 1.0)

        def scores_exp(m, u, j, ps, f_dve):
            q0 = QB * u
            sps = ps.tile([P, 2 * QB], F32, name=f"{pfx}sps{m}{u}{j}", tag="ps")
            nc.tensor.matmul(sps[:, 0:QB],
                             kT[m][0:D, P * j:P * (j + 1)],
                             qT[m][0:D, q0:q0 + QB],
                             start=True, stop=True, tile_position=(0, 0))
            nc.tensor.matmul(sps[:, QB:2 * QB],
                             kT[m][D:P, P * j:P * (j + 1)],
                             qT[m][D:P, q0:q0 + QB],
                             start=True, stop=True, tile_position=(64, 0))
            pt = work.tile([P, 2 * QB], BF16, name=f"{pfx}pt{m}{u}{j}", tag="pt")
            exp_acc[0] += f_dve
            if exp_acc[0] >= 1.0:
                exp_acc[0] -= 1.0
                nc.vector.tensor_scalar(pt[:].bitcast(I16), sps[:],
                                        EXP_A, EXP_B, ALU.mult, ALU.add)
            else:
                nc.scalar.activation(pt[:], sps[:], AF.Exp, scale=float(D) ** -0.5)
            return pt

        def ctx_accum(m, u, j, pt, cps):
            for g in range(2):
                nc.tensor.matmul(cps[:, QB * g:QB * (g + 1)],
                                 va[m][j][g][:],
                                 pt[:, QB * g:QB * (g + 1)],
                                 start=(j == 0), stop=(j == NT - 1))

        def normalize(m, u, cps):
            q0 = QB * u
            cu = norm.tile([D + 1, 2 * QB], F32, name=f"{pfx}cu{m}{u}", tag="cu")
            # split release of the cps bank: DVE half + ACT half run together
            nc.vector.tensor_copy(cu[:, 0:QB], cps[:, 0:QB])
            nc.scalar.copy(cu[:, QB:2 * QB], cps[:, QB:2 * QB])
            s_sb = norm.tile([1, 2 * QB], F32, name=f"{pfx}ssb{m}{u}", tag="ssb")
            nc.vector.tensor_copy(s_sb[:], cu[D:D + 1, :])
            rr = norm.tile([1, 2 * QB], F32, name=f"{pfx}rr{m}{u}", tag="rr")
            nc.vector.reciprocal_approx_fast(rr[:], s_sb[:])
            rb = norm.tile([D, 2 * QB], F32, name=f"{pfx}rb{m}{u}", tag="rb")
            nc.gpsimd.partition_broadcast(rb[:], rr[:])
            eng = nc.vector if m == MC - 1 else nc.gpsimd
            eng.tensor_mul(ctxT[m][0:D, q0:q0 + QB], cu[0:D, 0:QB], rb[:, 0:QB])
            eng.tensor_mul(ctxT[m][D:P, q0:q0 + QB], cu[0:D, QB:2 * QB], rb[:, QB:2 * QB])

        def outproj_t(t_i, ps):
            pso = ps.tile([P, C], F32, name=f"{pfx}pso{t_i}", tag="ps")
            for m in range(MC):
                nc.tensor.matmul(pso[:, 0:512], ctxT[m][:, P * t_i:P * (t_i + 1)],
                                 wo_b[m][:, 0:512], start=(m == 0), stop=(m == MC - 1))
                nc.tensor.matmul(pso[:, 512:C], ctxT[m][:, P * t_i:P * (t_i + 1)],
                                 wo_b[m][:, 512:C], start=(m == 0), stop=(m == MC - 1))
            ob = outp.tile([P, C], F32, name=f"{pfx}ob{t_i}", tag="ob")
            nc.vector.tensor_copy(ob[:], pso[:])
            nc.sync.dma_start(out[P * t_i:P * (t_i + 1), :], ob[:])

        def attn_block(m, u, ps, cpsp, f_dve, projs=(), outs=()):
            cps = cpsp.tile([D + 1, 2 * QB], F32, name=f"{pfx}cps{m}_{u}", tag="cps")
            pts = [scores_exp(m, u, 0, ps, f_dve), scores_exp(m, u, 1, ps, f_dve)]
            for fn in projs:
                fn()
            oi = 0
            for j in range(NT):
                if j + 2 < NT:
                    pts.append(scores_exp(m, u, j + 2, ps, f_dve))
                if oi < len(outs) and j in (5, 8, 11, 14):
                    outs[oi]()
                    oi += 1
                ctx_accum(m, u, j, pts[j], cps)
            normalize(m, u, cps)

        # ================= emission =================
        with tc.tile_pool(name=pfx + "xrp", bufs=4) as xrp, \
             tc.tile_pool(name=pfx + "xbp", bufs=5) as xbp, \
             tc.tile_pool(name=pfx + "wstage", bufs=3) as wstage, \
             tc.tile_pool(name=pfx + "ps", bufs=3, space="PSUM") as ps, \
             tc.tile_pool(name=pfx + "cps", bufs=1, space="PSUM") as cpsp:
            # weights for k/q first: they gate the first projections
            emit_weight_loads(wstage, [("k", wk), ("q", wq)])
            emit_xt_group(0, xrp, xbp, ps)
            cps00 = cpsp.tile([D + 1, 2 * QB], F32, name=f"{pfx}cps0_0", tag="cps")
            pts00 = []
            emit_proj_block("k", 0, 0, ps)
            emit_proj_block("q", 0, 0, ps)
            for j in range(4):
                pts00.append(scores_exp(0, 0, j, ps, F_DVE_LEAD))
            for tq in range(1, 4):
                emit_xt_group(tq, xrp, xbp, ps)
                if tq == 1:
                    emit_weight_loads(wstage, [("v", wv)])
                emit_proj_block("k", 0, tq, ps)
                emit_proj_block("q", 0, tq, ps)
                for j in range(4 * tq, 4 * (tq + 1)):
                    pts00.append(scores_exp(0, 0, j, ps, F_DVE_LEAD))
            for t_i in range(NT):
                emit_v_chunk(t_i, ps)
            emit_weight_loads(wstage, [("o", wo)])
            for j in range(NT):
                ctx_accum(0, 0, j, pts00[j], cps00)
            normalize(0, 0, cps00)

            K = lambda m, n: (lambda: emit_proj_block("k", m, n, ps))
            Q = lambda m, n: (lambda: emit_proj_block("q", m, n, ps))
            attn_block(0, 1, ps, cpsp, F_DVE_MAIN)
            attn_block(0, 2, ps, cpsp, F_DVE_MAIN,
                       projs=[K(1, 0), Q(1, 0), K(1, 1)])
            attn_block(0, 3, ps, cpsp, F_DVE_MAIN,
                       projs=[Q(1, 1), K(1, 2), Q(1, 2)])
            attn_block(1, 0, ps, cpsp, F_DVE_MAIN,
                       projs=[K(1, 3), Q(1, 3)])
            for u in range(1, NQ):
                attn_block(1, u, ps, cpsp, F_DVE_MAIN,
                           projs=[K(2, u - 1), Q(2, u - 1)])
            attn_block(2, 0, ps, cpsp, F_DVE_OUT,
                       projs=[K(2, 3), Q(2, 3)])
            for u in range(1, NQ):
                attn_block(2, u, ps, cpsp, F_DVE_OUT,
                           outs=[(lambda t=t: outproj_t(t, ps))
                                 for t in range(4 * (u - 1), 4 * u)])
            for t_i in range(4 * (NQ - 1), 4 * NQ):
                outproj_t(t_i, ps)

    with tile.TileContext(nc) as tc, ExitStack() as ctx:
        consts = ctx.enter_context(tc.tile_pool(name="consts", bufs=1))
        ident_f32 = consts.tile([P, P], mybir.dt.float32)
        make_identity(nc, ident_f32)
        ident_bf = consts.tile([P, P], BF16)
        nc.vector.tensor_copy(ident_bf[:], ident_f32[:])

        big = ctx.enter_context(tc.tile_pool(name="big", bufs=12))
        wrp = ctx.enter_context(tc.tile_pool(name="wrp", bufs=1))
        vap = ctx.enter_context(tc.tile_pool(name="vap", bufs=1))
        work = ctx.enter_context(tc.tile_pool(name="work", bufs=20))
        outp = ctx.enter_context(tc.tile_pool(name="outp", bufs=2))
        norm = ctx.enter_context(tc.tile_pool(name="norm", bufs=2))
        pools = (ident_bf, big, wrp, vap, work, outp, norm)
        for rep in range(repeat):
            emit(f"r{rep}_", tc, pools)

    nc.compile()
    return nc


def kernel(X, Wq, Wk, Wv, Wo, bo):
    from concourse import bass_utils

    if "nc" not in _cache:
        _cache["nc"] = _build(int(os.environ.get("KERNEL_REPEAT", "1")))
    nc = _cache["nc"]

    X = np.asarray(X, dtype=np.float32)
    in_maps = []
    for c in range(8):
        b, g = divmod(c, 2)
        sl = slice(HD * g, HD * (g + 1))
        in_maps.append({
            "x": np.ascontiguousarray(X[b]),
            "wq": np.ascontiguousarray(np.asarray(Wq, np.float32)[:, sl]),
            "wk": np.ascontiguousarray(np.asarray(Wk, np.float32)[:, sl]),
            "wv": np.ascontiguousarray(np.asarray(Wv, np.float32)[:, sl]),
            "wo": np.ascontiguousarray(np.asarray(Wo, np.float32)[sl, :]),
        })
    res = bass_utils.run_bass_kernel_spmd(nc, in_maps, core_ids=list(range(8)))
    _cache["last_result"] = res
    outf = np.empty((4, T, C), np.float32)
    bo = np.asarray(bo, np.float32)
    for b in range(4):
        outf[b] = res.results[2 * b]["out"] + res.results[2 * b + 1]["out"] + bo
    return outf


# revision 8
# speedup vs baseline: 1.0579x; 1.0579x over previous
"""Multi-head self-attention (B=4, T=2048, C=768, H=12) on 8 trn2 NeuronCores. v10.

Sharding: core c -> batch b=c//2, head-group g=c%2 (6 heads each).
Host sums the 2 partials per batch and adds the bias.

v10 (from v9): the attention steady state was gated by exp latency (ctx_j
waited ~1us on the ACT exp of chunk j; PE in-order queue head-of-line
blocked, DVFS dropped to mid p-state).
  - skew-2 emission: scores_{j+2} is emitted before ctx_j, so by the time
    the PE reaches ctx_j its exp has had ~2 iterations to finish.
  - one shared PSUM pool `ps` (3 bufs x [128,1024] f32 = 6 banks) serves
    scores chunks, projections, v chunks, transposes and outproj; cps
    keeps the last 2 banks. 3 score chunks in flight make the skew real.
  - exp alternates ACT/DVE (~60/40) via a fractional accumulator instead
    of the fixed 17% Schraudolph offload.
  - gpsimd (Pool engine, SBUF-only) absorbs x/weight casts and the
    normalize multiplies; DVE keeps PSUM-touching copies + its exp share.
  - cps released fast at block ends: cu copy halves go to DVE + ACT
    concurrently; proj/outproj bursts are emitted right after s0/s1 of
    the next block so the exp engines drain the backlog meanwhile.
"""
import sys
import os

sys.path.insert(0, "/opt/trn_rl_repo")

import numpy as np

P = 128
T = 2048
C = 768
HD = 384          # per-core head columns (6 heads x 64)
D = 64
NT = T // P       # 16 key chunks of 128
KC = C // P       # 6 contraction chunks for C
MC = HD // P      # 3 head pairs
QB = 512          # query block
NQ = T // QB      # 4 query blocks

EXP_A = 128 * 0.125 * float(np.log2(np.e))   # 23.083120654223414
EXP_B = 128 * 127 - 5.5                      # 16250.5 (Schraudolph bias, bf16)

# DVE share of exp chunks per phase (fractional accumulator)
F_DVE_LEAD = 0.34    # block (0,0): DVE busy with lead-in casts/copies
F_DVE_MAIN = 0.40    # blocks m=0 u>=1 .. m=1
F_DVE_OUT = 0.30     # m=2 phase: DVE also does outproj copies

_cache = {}


def _build(repeat=1):
    import concourse.bacc as bacc
    import concourse.mybir as mybir
    import concourse.tile as tile
    from concourse.masks import make_identity
    from contextlib import ExitStack

    F32 = mybir.dt.float32
    BF16 = mybir.dt.bfloat16
    I16 = mybir.dt.int16
    AF = mybir.ActivationFunctionType
    ALU = mybir.AluOpType

    nc = bacc.Bacc("TRN2", target_bir_lowering=False, debug=False)
    x = nc.dram_tensor("x", [T, C], F32, kind="ExternalInput").ap()
    wq = nc.dram_tensor("wq", [C, HD], F32, kind="ExternalInput").ap()
    wk = nc.dram_tensor("wk", [C, HD], F32, kind="ExternalInput").ap()
    wv = nc.dram_tensor("wv", [C, HD], F32, kind="ExternalInput").ap()
    wo = nc.dram_tensor("wo", [HD, C], F32, kind="ExternalInput").ap()
    out = nc.dram_tensor("out", [T, C], F32, kind="ExternalOutput").ap()

    def emit(pfx, tc, pools):
        ident_bf, big, wrp, vap, work, outp, norm = pools

        xt = [big.tile([P, T], BF16, name=f"{pfx}xt{kc}", tag="big2048") for kc in range(KC)]
        qT = [big.tile([P, T], BF16, name=f"{pfx}qT{m}", tag="big2048") for m in range(MC)]
        kT = [big.tile([P, T], BF16, name=f"{pfx}kT{m}", tag="big2048") for m in range(MC)]
        ctxT = [big.tile([P, T], BF16, name=f"{pfx}ctxT{m}", tag="big2048") for m in range(MC)]
        va = [[[vap.tile([P, D + 1], BF16, name=f"{pfx}va{m}_{t}_{g}", tag=f"va{m}_{t}_{g}")
                for g in range(2)] for t in range(NT)] for m in range(MC)]

        w_b = {}
        wo_b = []
        exp_acc = [0.0]

        def emit_weight_loads(wstage, names):
            for nm, src in names:
                if nm == "o":
                    for m in range(MC):
                        st = wstage.tile([P, C], F32, name=f"{pfx}wst_o{m}", tag="wsto")
                        nc.sync.dma_start(st[:], wo[P * m:P * (m + 1), :])
                        t_b = wrp.tile([P, C], BF16, name=f"{pfx}wo_{m}", tag=f"wo_{m}")
                        nc.gpsimd.tensor_copy(t_b[:], st[:])
                        wo_b.append(t_b)
                    continue
                for kc in range(KC):
                    st = wstage.tile([P, HD], F32, name=f"{pfx}wst_{nm}{kc}", tag="wst")
                    nc.sync.dma_start(st[:], src[P * kc:P * (kc + 1), :])
                    t_b = wrp.tile([P, HD], BF16, name=f"{pfx}w_{nm}{kc}", tag=f"w_{nm}{kc}")
                    # k/q gate the first projections: fast DVE casts; v is
                    # off the critical path: slow-but-idle gpsimd
                    if nm == "v":
                        nc.gpsimd.tensor_copy(t_b[:], st[:])
                    else:
                        nc.vector.tensor_copy(t_b[:], st[:])
                    w_b[nm, kc] = t_b

        def emit_xt_group(tq, xrp, xbp, ps):
            xbs = []
            for i in range(4):
                t_i = 4 * tq + i
                xr = xrp.tile([P, C], F32, name=f"{pfx}xr{t_i}", tag="xr")
                nc.sync.dma_start(xr[:], x[P * t_i:P * (t_i + 1), :])
                xb = xbp.tile([P, C], BF16, name=f"{pfx}xb{t_i}", tag="xb")
                nc.vector.tensor_copy(xb[:], xr[:])
                xbs.append(xb)
            for kc in range(KC):
                tp = ps.tile([P, 512], BF16, name=f"{pfx}tp_{tq}_{kc}", tag="ps")
                for i in range(4):
                    nc.tensor.transpose(tp[:, P * i:P * (i + 1)],
                                        xbs[i][:, P * kc:P * (kc + 1)], ident_bf[:])
                nc.vector.tensor_copy(xt[kc][:, 512 * tq:512 * (tq + 1)], tp[:])

        def emit_proj_block(nm, m, n, ps):
            dest = qT if nm == "q" else kT
            pst = ps.tile([P, 512], F32, name=f"{pfx}ps_{nm}{m}{n}", tag="ps")
            for kc in range(KC):
                nc.tensor.matmul(
                    pst[:],
                    w_b[nm, kc][:, P * m:P * (m + 1)],
                    xt[kc][:, 512 * n:512 * (n + 1)],
                    start=(kc == 0), stop=(kc == KC - 1),
                )
            if m == 0:
                nc.scalar.copy(dest[m][:, 512 * n:512 * (n + 1)], pst[:])
            else:
                nc.vector.tensor_copy(dest[m][:, 512 * n:512 * (n + 1)], pst[:])

        def emit_v_chunk(t_i, ps):
            pv = ps.tile([P, HD], F32, name=f"{pfx}pv{t_i}", tag="ps")
            for kc in range(KC):
                nc.tensor.matmul(
                    pv[:],
                    xt[kc][:, P * t_i:P * (t_i + 1)],
                    w_b["v", kc][:],
                    start=(kc == 0), stop=(kc == KC - 1),
                )
            for m in range(MC):
                for g in range(2):
                    vt = va[m][t_i][g]
                    nc.vector.tensor_copy(vt[:, 0:D], pv[:, P * m + D * g:P * m + D * (g + 1)])
                    nc.gpsimd.memset(vt[:, D:D + 1], 1.0)

        def scores_exp(m, u, j, ps, f_dve):
            q0 = QB * u
            sps = ps.tile([P, 2 * QB], F32, name=f"{pfx}sps{m}{u}{j}", tag="ps")
            nc.tensor.matmul(sps[:, 0:QB],
                             kT[m][0:D, P * j:P * (j + 1)],
                             qT[m][0:D, q0:q0 + QB],
                             start=True, stop=True, tile_position=(0, 0))
            nc.tensor.matmul(sps[:, QB:2 * QB],
                             kT[m][D:P, P * j:P * (j + 1)],
                             qT[m][D:P, q0:q0 + QB],
                             start=True, stop=True, tile_position=(64, 0))
            pt = work.tile([P, 2 * QB], BF16, name=f"{pfx}pt{m}{u}{j}", tag="pt")
            exp_acc[0] += f_dve
            if exp_acc[0] >= 1.0:
                exp_acc[0] -= 1.0
                nc.vector.tensor_scalar(pt[:].bitcast(I16), sps[:],
                                        EXP_A, EXP_B, ALU.mult, ALU.add)
            else:
                nc.scalar.activation(pt[:], sps[:], AF.Exp, scale=float(D) ** -0.5)
            return pt

        def ctx_accum(m, u, j, pt, cps):
            for g in range(2):
                nc.tensor.matmul(cps[:, QB * g:QB * (g + 1)],
                                 va[m][j][g][:],
                                 pt[:, QB * g:QB * (g + 1)],
                                 start=(j == 0), stop=(j == NT - 1))

        def normalize(m, u, cps):
            q0 = QB * u
            cu = norm.tile([D + 1, 2 * QB], F32, name=f"{pfx}cu{m}{u}", tag="cu")
            # split release of the cps bank: DVE half + ACT half run together
            nc.vector.tensor_copy(cu[:, 0:QB], cps[:, 0:QB])
            nc.scalar.copy(cu[:, QB:2 * QB], cps[:, QB:2 * QB])
            s_sb = norm.tile([1, 2 * QB], F32, name=f"{pfx}ssb{m}{u}", tag="ssb")
            nc.vector.tensor_copy(s_sb[:], cu[D:D + 1, :])
            rr = norm.tile([1, 2 * QB], F32, name=f"{pfx}rr{m}{u}", tag="rr")
            nc.vector.reciprocal_approx_fast(rr[:], s_sb[:])
            rb = norm.tile([D, 2 * QB], F32, name=f"{pfx}rb{m}{u}", tag="rb")
            nc.gpsimd.partition_broadcast(rb[:], rr[:])
            nc.gpsimd.tensor_mul(ctxT[m][0:D, q0:q0 + QB], cu[0:D, 0:QB], rb[:, 0:QB])
            nc.gpsimd.tensor_mul(ctxT[m][D:P, q0:q0 + QB], cu[0:D, QB:2 * QB], rb[:, QB:2 * QB])

        def outproj_t(t_i, ps):
            pso = ps.tile([P, C], F32, name=f"{pfx}pso{t_i}", tag="ps")
            for m in range(MC):
                nc.tensor.matmul(pso[:, 0:512], ctxT[m][:, P * t_i:P * (t_i + 1)],
                                 wo_b[m][:, 0:512], start=(m == 0), stop=(m == MC - 1))
                nc.tensor.matmul(pso[:, 512:C], ctxT[m][:, P * t_i:P * (t_i + 1)],
                                 wo_b[m][:, 512:C], start=(m == 0), stop=(m == MC - 1))
            ob = outp.tile([P, C], F32, name=f"{pfx}ob{t_i}", tag="ob")
            nc.vector.tensor_copy(ob[:], pso[:])
            nc.sync.dma_start(out[P * t_i:P * (t_i + 1), :], ob[:])

        def attn_block(m, u, ps, cpsp, f_dve, projs=(), outs=()):
            # skew-2, 2-step grouped emission: [s_{j+2}, s_{j+3}] then
            # [c_j, c_{j+1}] — halves the PE 64<->128 tile-config switches
            # (an unhidden LDWEIGHTS, ~135ns) and keeps the exp engines 2-3
            # chunks ahead of the ctx consumer. Hooks (projections/outproj
            # bursts) land between a scores group and its ctx group, deep in
            # the block where the exp backlog can absorb the PE detour.
            cps = cpsp.tile([D + 1, 2 * QB], F32, name=f"{pfx}cps{m}_{u}", tag="cps")
            hooks = {}
            for i, fn in enumerate(projs):
                hooks.setdefault({0: 4, 1: 8, 2: 12}[i], []).append(fn)
            for i, fn in enumerate(outs):
                hooks.setdefault({0: 2, 1: 6, 2: 10, 3: 14}[i], []).append(fn)
            pts = [scores_exp(m, u, 0, ps, f_dve), scores_exp(m, u, 1, ps, f_dve)]
            for j in range(0, NT, 2):
                if j + 2 < NT:
                    pts.append(scores_exp(m, u, j + 2, ps, f_dve))
                if j + 3 < NT:
                    pts.append(scores_exp(m, u, j + 3, ps, f_dve))
                for fn in hooks.get(j, ()):
                    fn()
                ctx_accum(m, u, j, pts[j], cps)
                ctx_accum(m, u, j + 1, pts[j + 1], cps)
            normalize(m, u, cps)

        # ================= emission =================
        with tc.tile_pool(name=pfx + "xrp", bufs=4) as xrp, \
             tc.tile_pool(name=pfx + "xbp", bufs=5) as xbp, \
             tc.tile_pool(name=pfx + "wstage", bufs=3) as wstage, \
             tc.tile_pool(name=pfx + "ps", bufs=3, space="PSUM") as ps, \
             tc.tile_pool(name=pfx + "cps", bufs=1, space="PSUM") as cpsp:
            # weights for k/q first: they gate the first projections
            emit_weight_loads(wstage, [("k", wk), ("q", wq)])
            emit_xt_group(0, xrp, xbp, ps)
            cps00 = cpsp.tile([D + 1, 2 * QB], F32, name=f"{pfx}cps0_0", tag="cps")
            pts00 = []
            emit_proj_block("k", 0, 0, ps)
            emit_proj_block("q", 0, 0, ps)
            for j in range(4):
                pts00.append(scores_exp(0, 0, j, ps, F_DVE_LEAD))
            for tq in range(1, 4):
                emit_xt_group(tq, xrp, xbp, ps)
                if tq == 1:
                    emit_weight_loads(wstage, [("v", wv)])
                emit_proj_block("k", 0, tq, ps)
                emit_proj_block("q", 0, tq, ps)
                for j in range(4 * tq, 4 * (tq + 1)):
                    pts00.append(scores_exp(0, 0, j, ps, F_DVE_LEAD))
            for t_i in range(NT):
                emit_v_chunk(t_i, ps)
            emit_weight_loads(wstage, [("o", wo)])
            for j in range(NT):
                ctx_accum(0, 0, j, pts00[j], cps00)
            normalize(0, 0, cps00)

            K = lambda m, n: (lambda: emit_proj_block("k", m, n, ps))
            Q = lambda m, n: (lambda: emit_proj_block("q", m, n, ps))
            attn_block(0, 1, ps, cpsp, F_DVE_MAIN)
            attn_block(0, 2, ps, cpsp, F_DVE_MAIN,
                       projs=[K(1, 0), Q(1, 0), K(1, 1)])
            attn_block(0, 3, ps, cpsp, F_DVE_MAIN,
                       projs=[Q(1, 1), K(1, 2), Q(1, 2)])
            attn_block(1, 0, ps, cpsp, F_DVE_MAIN,
                       projs=[K(1, 3), Q(1, 3)])
            for u in range(1, NQ):
                attn_block(1, u, ps, cpsp, F_DVE_MAIN,
                           projs=[K(2, u - 1), Q(2, u - 1)])
            attn_block(2, 0, ps, cpsp, F_DVE_OUT,
                       projs=[K(2, 3), Q(2, 3)])
            for u in range(1, NQ):
                attn_block(2, u, ps, cpsp, F_DVE_OUT,
                           outs=[(lambda t=t: outproj_t(t, ps))
                                 for t in range(4 * (u - 1), 4 * u)])
            for t_i in range(4 * (NQ - 1), 4 * NQ):
                outproj_t(t_i, ps)

    with tile.TileContext(nc) as tc, ExitStack() as ctx:
        consts = ctx.enter_context(tc.tile_pool(name="consts", bufs=1))
        ident_f32 = consts.tile([P, P], mybir.dt.float32)
        make_identity(nc, ident_f32)
        ident_bf = consts.tile([P, P], BF16)
        nc.vector.tensor_copy(ident_bf[:], ident_f32[:])

        big = ctx.enter_context(tc.tile_pool(name="big", bufs=12))
        wrp = ctx.enter_context(tc.tile_pool(name="wrp", bufs=1))
        vap = ctx.enter_context(tc.tile_pool(name="vap", bufs=1))
        work = ctx.enter_context(tc.tile_pool(name="work", bufs=20))
        outp = ctx.enter_context(tc.tile_pool(name="outp", bufs=2))
        norm = ctx.enter_context(tc.tile_pool(name="norm", bufs=2))
        pools = (ident_bf, big, wrp, vap, work, outp, norm)
        for rep in range(repeat):
            emit(f"r{rep}_", tc, pools)

    nc.compile()
    return nc


def kernel(X, Wq, Wk, Wv, Wo, bo):
    from concourse import bass_utils

    if "nc" not in _cache:
        _cache["nc"] = _build(int(os.environ.get("KERNEL_REPEAT", "1")))
    nc = _cache["nc"]

    X = np.asarray(X, dtype=np.float32)
    in_maps = []
    for c in range(8):
        b, g = divmod(c, 2)
        sl = slice(HD * g, HD * (g + 1))
        in_maps.append({
            "x": np.ascontiguousarray(X[b]),
            "wq": np.ascontiguousarray(np.asarray(Wq, np.float32)[:, sl]),
            "wk": np.ascontiguousarray(np.asarray(Wk, np.float32)[:, sl]),
            "wv": np.ascontiguousarray(np.asarray(Wv, np.float32)[:, sl]),
            "wo": np.ascontiguousarray(np.asarray(Wo, np.float32)[sl, :]),
        })
    res = bass_utils.run_bass_kernel_spmd(nc, in_maps, core_ids=list(range(8)))
    _cache["last_result"] = res
    outf = np.empty((4, T, C), np.float32)
    bo = np.asarray(bo, np.float32)
    for b in range(4):
        outf[b] = res.results[2 * b]["out"] + res.results[2 * b + 1]["out"] + bo
    return outf
